# revision 1
# baseline (speedup 1.0000x reference)
"""DigitCaps dynamic-routing kernel for 8 TRN2 NeuronCores.

Reference computation (f32):
    u_hat[b,r,j,d] = sum_c W[r,j,d,c] * x[b,r,c]
    b_ij = 0
    for it in 1..3:
        c = softmax_j(b_ij)                       # [R, J]
        s[b,j,d] = sum_r c[r,j] u_hat[b,r,j,d]
        v = squash(s)                             # [B, J, D]
        b_ij += mean_b sum_d u_hat[b,r,j,d] v[b,j,d]
    return v[..., None]

Strategy: shard the R=1152 routes across 8 cores (144 each), full batch on
every core.  u_hat is never materialized; both big contractions go through
the rank-8 factorization:
    s[b,jd]   = sum_{rc} x[b,(rc)] * (c[r,j] W[(rc),(jd)])     (PE matmuls)
    a[r,j]    = sum_{cd} W[(rc),(jd)] * Q[(rc),(jd)]           (DVE + PE)
      where Q[(rc),(jd)] = (1/B) sum_b x[b,(rc)] v[b,(jd)]     (PE matmuls)
Per-iteration collective: one 80KB AllReduce of the partial s.
Host pre-arranges all device layouts so every DMA is contiguous.

All big matmuls run as compensated bf16 3-pass (hi*hi + lo*hi + hi*lo):
fp32 matmul on TRN2 is 4 cycles/row (LOW_HIGH dual pass) with slow
non-FWL weight loads; the bf16 triple runs >3x faster at ~1e-5 rel error.
"""

import sys

sys.path.insert(0, "/opt/trn_rl_repo")

import ml_dtypes
import numpy as np

B = 128          # batch
R = 1152         # num_routes
J = 10           # num_caps
D = 16           # caps_dim
C = 8            # caps_in
NUM_IT = 3
N_CORES = 8
RG = 9           # route groups per core (16 routes x 8 c = 128 partitions)
JD = J * D       # 160

_CACHE = {}

TRACE = False  # set True (e.g. from test.py) to capture HW profile/exec time
TRACE_DIR = None
PRECISION = "bf16x3"  # "bf16x3" (safe, ~1e-5) or "f16" (fast, ~2e-4)


def _build_bass(precision="bf16x3"):
    import concourse.bacc as bacc
    import concourse.mybir as mybir
    from concourse.tile import TileContext

    f32 = mybir.dt.float32
    bf16 = mybir.dt.bfloat16
    f16 = mybir.dt.float16
    F16 = precision == "f16"
    mdt = f16 if F16 else bf16
    nc = bacc.Bacc(None, target_bir_lowering=False, num_devices=N_CORES)

    if F16:
        xrc_hi = nc.dram_tensor("xrc16", [128, RG * B], f16,
                                kind="ExternalInput")
        xt_hi = nc.dram_tensor("xt16", [B, RG * 128], f16,
                               kind="ExternalInput")
        w = nc.dram_tensor("w16", [128, RG * JD], f16, kind="ExternalInput")
        xrc_lo = xt_lo = None
    else:
        xrc_hi = nc.dram_tensor("xrc_hi", [128, RG * B], bf16,
                                kind="ExternalInput")
        xrc_lo = nc.dram_tensor("xrc_lo", [128, RG * B], bf16,
                                kind="ExternalInput")
        xt_hi = nc.dram_tensor("xt_hi", [B, RG * 128], bf16,
                               kind="ExternalInput")
        xt_lo = nc.dram_tensor("xt_lo", [B, RG * 128], bf16,
                               kind="ExternalInput")
        w = nc.dram_tensor("wh_d", [128, RG * JD], bf16, kind="ExternalInput")
        wl_d = nc.dram_tensor("wl_d", [128, RG * JD], bf16,
                              kind="ExternalInput")
    bpat = nc.dram_tensor("bpat", [128, 128], f32, kind="ExternalInput")
    BS = B // N_CORES  # final-iteration batch shard per core
    out = nc.dram_tensor("out", [BS, JD], f32, kind="ExternalOutput")

    # collective bounce buffers (DRAM)
    s_in = nc.dram_tensor("s_in", [B, JD], f32)
    s_out = nc.dram_tensor("s_out", [B, JD], f32, addr_space="Shared")
    s3_out = nc.dram_tensor("s3_out", [BS, JD], f32)

    groups = [list(range(N_CORES))]

    with TileContext(nc) as tc:
        with (
            tc.tile_pool(name="inp", bufs=1) as inp,
            tc.tile_pool(name="work", bufs=2) as work,
            tc.tile_pool(name="small", bufs=1) as small,
            tc.tile_pool(name="psum", bufs=2, space="PSUM") as psum,
            tc.tile_pool(name="apsum", bufs=3, space="PSUM") as apsum,
            tc.tile_pool(name="qpsum", bufs=3, space="PSUM") as qpsum,
        ):
            xrch_sb = inp.tile([128, RG * B], mdt, tag="xrch")
            xth_sb = inp.tile([B, RG * 128], mdt, tag="xth")
            bpat_sb = inp.tile([128, 128], f32, tag="bpat")
            if F16:
                xrcl_sb = xtl_sb = None
                w_sb = inp.tile([128, RG * JD], f16, tag="w")
                wh_sb = w_sb
                wl_sb = None
            else:
                xrcl_sb = inp.tile([128, RG * B], bf16, tag="xrcl")
                xtl_sb = inp.tile([B, RG * 128], bf16, tag="xtl")
                w_sb = small.tile([128, RG * JD], f32, tag="w")
                wh_sb = inp.tile([128, RG * JD], bf16, tag="wh")
                wl_sb = inp.tile([128, RG * JD], bf16, tag="wl")

            # per-group-chunk loads of xrc/w so iteration-1 matmuls start
            # while the rest of the inputs stream in; xt/bpat are not needed
            # until after the first AllReduce and get emitted later.
            for g3 in range(3):  # 3-group chunks: larger DMA runs, still piped
                gs, ge = g3 * 3, (g3 + 1) * 3
                nc.sync.dma_start(
                    out=xrch_sb[:, gs * B:ge * B],
                    in_=xrc_hi[:, gs * B:ge * B])
                if not F16:
                    nc.sync.dma_start(
                        out=xrcl_sb[:, gs * B:ge * B],
                        in_=xrc_lo[:, gs * B:ge * B])
                nc.sync.dma_start(
                    out=wh_sb[:, gs * JD:ge * JD],
                    in_=w[:, gs * JD:ge * JD])
                if not F16:
                    nc.sync.dma_start(
                        out=wl_sb[:, gs * JD:ge * JD],
                        in_=wl_d[:, gs * JD:ge * JD])

            junk_sb = small.tile([1, 1], f32, tag="junk")
            nc.vector.memset(junk_sb[:], 1.0)
            # pre-load the ACT Sqrt table while the input DMA streams, so the
            # first squash's Sqrt doesn't pay the 1.3us table load on-chain
            nc.scalar.activation(out=junk_sb[:], in_=junk_sb[:],
                                 func=mybir.ActivationFunctionType.Sqrt)

            cw_sb = small.tile([128, RG * JD], f32, tag="cw")
            cwh_sb = small.tile([128, RG * JD], mdt, tag="cwh")
            cwl_sb = (None if F16
                      else small.tile([128, RG * JD], bf16, tag="cwl"))
            b_sb = small.tile([128, RG * J], f32, tag="bij")
            c_sb = small.tile([128, RG * J], f32, tag="cij")
            t_sb = small.tile([128, RG * J], f32, tag="t")
            v_sb = small.tile([B, JD], f32, tag="v")
            vh_sb = small.tile([B, JD], mdt, tag="vh")
            vl_sb = None if F16 else small.tile([B, JD], bf16, tag="vl")
            s_sb = small.tile([B, JD], f32, tag="s")
            # softmax/squash temporaries
            m_sb = small.tile([128, RG], f32, tag="m")
            e_sb = small.tile([128, RG * J], f32, tag="e")
            sqn_sb = small.tile([B, J], f32, tag="sqn")
            rt_sb = small.tile([B, J], f32, tag="rt")
            den_sb = small.tile([B, J], f32, tag="den")
            fac_sb = small.tile([B, J], f32, tag="fac")
            s2_sb = small.tile([B, JD], f32, tag="s2")

            def softmax():
                """c_sb = softmax_j(b_sb), per (partition, g)."""
                cv = c_sb[:].rearrange("p (g j) -> p g j", g=RG, j=J)
                ev = e_sb[:].rearrange("p (g j) -> p g j", g=RG, j=J)
                # logits are bounded (|b| < ~10): exp without max-shift
                nc.scalar.activation(
                    out=e_sb[:], in_=b_sb[:], func=mybir.ActivationFunctionType.Exp
                )
                # hoist the next squash's Sqrt table load off the chain
                nc.scalar.activation(out=junk_sb[:], in_=e_sb[0:1, 0:1],
                                     func=mybir.ActivationFunctionType.Sqrt)
                nc.vector.tensor_reduce(
                    out=m_sb[:], in_=ev, axis=mybir.AxisListType.X,
                    op=mybir.AluOpType.add,
                )
                nc.vector.reciprocal(out=m_sb[:], in_=m_sb[:])
                rb = m_sb[:].unsqueeze(-1).broadcast_to([128, RG, J])
                nc.vector.tensor_tensor(
                    out=cv, in0=ev, in1=rb, op=mybir.AluOpType.mult
                )

            def compute_cw():
                """cw = w * c (broadcast over d), split hi/lo bf16."""
                wv = w_sb[:].rearrange("p (g j d) -> p g j d", g=RG, j=J, d=D)
                cwv = cw_sb[:].rearrange("p (g j d) -> p g j d", g=RG, j=J, d=D)
                cb = (
                    c_sb[:]
                    .rearrange("p (g j) -> p g j", g=RG, j=J)
                    .unsqueeze(-1)
                    .broadcast_to([128, RG, J, D])
                )
                if F16:
                    cwhv = cwh_sb[:].rearrange("p (g j d) -> p g j d",
                                               g=RG, j=J, d=D)
                    nc.vector.tensor_tensor(out=cwhv, in0=wv, in1=cb,
                                            op=mybir.AluOpType.mult)
                else:
                    # chunked by 3-group slices: the s-matmuls on early
                    # groups start while later chunks are still built
                    cwhc = cwh_sb[:].rearrange("p (g f) -> p g f", g=RG, f=JD)
                    cwlc = cwl_sb[:].rearrange("p (g f) -> p g f", g=RG, f=JD)
                    cwc = cw_sb[:].rearrange("p (g f) -> p g f", g=RG, f=JD)
                    for g3 in range(3):
                        gs, ge = g3 * 3, (g3 + 1) * 3
                        nc.vector.tensor_tensor(
                            out=cwv[:, gs:ge], in0=wv[:, gs:ge],
                            in1=cb[:, gs:ge], op=mybir.AluOpType.mult)
                        nc.scalar.copy(out=cwhc[:, gs:ge, :],
                                       in_=cwc[:, gs:ge, :])
                        nc.vector.tensor_tensor(
                            out=cwlc[:, gs:ge, :], in0=cwc[:, gs:ge, :],
                            in1=cwhc[:, gs:ge, :],
                            op=mybir.AluOpType.subtract)

            def s_matmuls(rhsh_sb, rhsl_sb):
                """PSUM-accumulated compensated product sum over all groups."""
                s_ps = psum.tile([128, JD], f32, tag="s_ps")
                xh = xrch_sb[:].rearrange("p (g b) -> p g b", g=RG, b=B)
                rh = rhsh_sb[:].rearrange("p (g f) -> p g f", g=RG, f=JD)
                if F16:
                    terms = lambda g: ((xh[:, g, :], rh[:, g, :]),)
                    n = RG
                else:
                    xl = xrcl_sb[:].rearrange("p (g b) -> p g b", g=RG, b=B)
                    rl = rhsl_sb[:].rearrange("p (g f) -> p g f", g=RG, f=JD)
                    terms = lambda g: (
                        (xh[:, g, :], rh[:, g, :]),
                        (xh[:, g, :], rl[:, g, :]),
                        (xl[:, g, :], rh[:, g, :]),
                    )
                    n = 3 * RG
                i = 0
                for g in range(RG):
                    for lhs, rhs in terms(g):
                        nc.tensor.matmul(
                            s_ps[:], lhsT=lhs, rhs=rhs,
                            start=(i == 0), stop=(i == n - 1),
                        )
                        i += 1
                return s_ps

            def squash(scale, P=B, split_v=True):
                """v_sb[:P] = squash(scale * s_sb[:P]); optional hi/lo split."""
                s_ap = s_sb[0:P, :]
                v_ap = v_sb[0:P, :]
                vv = v_ap.rearrange("b (j d) -> b j d", j=J, d=D)
                s2v = s2_sb[0:P, :].rearrange("b (j d) -> b j d", j=J, d=D)
                # s <- scale * s  (ACT Copy, no table)
                if scale != 1.0:
                    nc.scalar.mul(out=s_ap, in_=s_ap, mul=float(scale))
                nc.vector.tensor_tensor(out=s2_sb[0:P, :], in0=s_ap, in1=s_ap,
                                        op=mybir.AluOpType.mult)
                nc.vector.tensor_reduce(
                    out=sqn_sb[0:P, :], in_=s2v, axis=mybir.AxisListType.X,
                    op=mybir.AluOpType.add,
                )
                # factor = sqrt(sqn) / (1 + sqn)
                nc.scalar.activation(
                    out=rt_sb[0:P, :], in_=sqn_sb[0:P, :],
                    func=mybir.ActivationFunctionType.Sqrt,
                )
                if split_v:
                    # hoist the next softmax's Exp table load off the chain
                    nc.scalar.activation(out=junk_sb[:], in_=rt_sb[0:1, 0:1],
                                         func=mybir.ActivationFunctionType.Exp)
                nc.vector.tensor_scalar_add(out=den_sb[0:P, :],
                                            in0=sqn_sb[0:P, :], scalar1=1.0)
                nc.vector.reciprocal(out=den_sb[0:P, :], in_=den_sb[0:P, :])
                nc.vector.tensor_tensor(out=fac_sb[0:P, :], in0=rt_sb[0:P, :],
                                        in1=den_sb[0:P, :],
                                        op=mybir.AluOpType.mult)
                fb = fac_sb[0:P, :].unsqueeze(-1).broadcast_to([P, J, D])
                sv = s_ap.rearrange("b (j d) -> b j d", j=J, d=D)
                nc.vector.tensor_tensor(out=vv, in0=sv, in1=fb,
                                        op=mybir.AluOpType.mult)
                if split_v:
                    vhv = vh_sb[:].rearrange("b (j d) -> b j d", j=J, d=D)
                    nc.vector.tensor_tensor(out=vhv, in0=sv, in1=fb,
                                            op=mybir.AluOpType.mult)
                    if not F16:
                        nc.vector.tensor_tensor(out=vl_sb[:], in0=v_sb[:],
                                                in1=vh_sb[:],
                                                op=mybir.AluOpType.subtract)

            def agreement(first):
                """t_sb[p,(g,j)] = sum_d W*Q;  b_sb += bpat^T @ t (c-sum,
                broadcast over c partitions, * 1/B folded into bpat)."""
                wv = w_sb[:].rearrange("p (g f) -> p g f", g=RG, f=JD)
                tv = t_sb[:].rearrange("p (g j) -> p g j", g=RG, j=J)
                for g in range(RG):
                    q_ps = qpsum.tile([128, JD], f32, tag="q_ps")
                    if F16:
                        qterms = ((xth_sb[:, g * 128:(g + 1) * 128],
                                   vh_sb[:]),)
                    else:
                        qterms = (
                            (xth_sb[:, g * 128:(g + 1) * 128], vh_sb[:]),
                            (xth_sb[:, g * 128:(g + 1) * 128], vl_sb[:]),
                            (xtl_sb[:, g * 128:(g + 1) * 128], vh_sb[:]),
                        )
                    for i, (lhs, rhs) in enumerate(qterms):
                        nc.tensor.matmul(q_ps[:], lhsT=lhs, rhs=rhs,
                                         start=(i == 0),
                                         stop=(i == len(qterms) - 1))
                    prod = work.tile([128, JD], f32, tag="prod")
                    pv = prod[:].rearrange("p (j d) -> p j d", j=J, d=D)
                    nc.vector.tensor_tensor(
                        out=prod[:], in0=wv[:, g, :],
                        in1=q_ps[:], op=mybir.AluOpType.mult,
                    )
                    nc.vector.tensor_reduce(
                        out=tv[:, g, :], in_=pv, axis=mybir.AxisListType.X,
                        op=mybir.AluOpType.add,
                    )
                a_ps = apsum.tile([128, RG * J], f32, tag="a_ps")
                nc.tensor.matmul(a_ps[:], lhsT=bpat_sb[:], rhs=t_sb[:],
                                 start=True, stop=True)
                if first:
                    nc.vector.tensor_copy(out=b_sb[:], in_=a_ps[:])
                else:
                    nc.vector.tensor_tensor(out=b_sb[:], in0=b_sb[:],
                                            in1=a_ps[:],
                                            op=mybir.AluOpType.add)

            def fused_step(first):
                """Agreement + b-update + softmax + cw + s-matmuls, pipelined
                in 3-group chunks: the a-matmul/softmax/cw/s-MMs of early
                chunks run while later chunks' Q-matmuls are still going."""
                s_ps = psum.tile([128, JD], f32, tag="s_ps")
                wv = w_sb[:].rearrange("p (g f) -> p g f", g=RG, f=JD)
                wv4 = w_sb[:].rearrange("p (g j d) -> p g j d", g=RG, j=J, d=D)
                tv = t_sb[:].rearrange("p (g j) -> p g j", g=RG, j=J)
                ev3 = e_sb[:].rearrange("p (g j) -> p g j", g=RG, j=J)
                cv3 = c_sb[:].rearrange("p (g j) -> p g j", g=RG, j=J)
                cwv4 = cw_sb[:].rearrange("p (g j d) -> p g j d",
                                          g=RG, j=J, d=D)
                cwh3 = cwh_sb[:].rearrange("p (g f) -> p g f", g=RG, f=JD)
                cb4 = (c_sb[:].rearrange("p (g j) -> p g j", g=RG, j=J)
                       .unsqueeze(-1).broadcast_to([128, RG, J, D]))
                xh = xrch_sb[:].rearrange("p (g b) -> p g b", g=RG, b=B)
                if not F16:
                    cwl3 = cwl_sb[:].rearrange("p (g f) -> p g f", g=RG, f=JD)
                    xl = xrcl_sb[:].rearrange("p (g b) -> p g b", g=RG, b=B)
                    n_s = 3 * RG
                else:
                    n_s = RG
                i = 0
                for g3 in range(3):
                    gs, ge = 3 * g3, 3 * (g3 + 1)
                    # one wide PSUM tile per chunk: the W*Q multiply and
                    # d-reduce then run as 2 DVE ops instead of 6
                    q_ps = qpsum.tile([128, 3 * JD], f32, tag="q_ps")
                    for g in range(gs, ge):
                        off = (g - gs) * JD
                        if F16:
                            qterms = ((xth_sb[:, g * 128:(g + 1) * 128],
                                       vh_sb[:]),)
                        else:
                            qterms = (
                                (xth_sb[:, g * 128:(g + 1) * 128], vh_sb[:]),
                                (xth_sb[:, g * 128:(g + 1) * 128], vl_sb[:]),
                                (xtl_sb[:, g * 128:(g + 1) * 128], vh_sb[:]),
                            )
                        for qi, (lhs, rhs) in enumerate(qterms):
                            nc.tensor.matmul(q_ps[:, off:off + JD],
                                             lhsT=lhs, rhs=rhs,
                                             start=(qi == 0),
                                             stop=(qi == len(qterms) - 1),
                                             skip_group_check=True)
                    prod = work.tile([128, 3 * JD], f32, tag="prod")
                    pv = prod[:].rearrange("p (gj d) -> p gj d",
                                           gj=3 * J, d=D)
                    nc.vector.tensor_tensor(
                        out=prod[:], in0=w_sb[:, gs * JD:ge * JD],
                        in1=q_ps[:], op=mybir.AluOpType.mult)
                    nc.vector.tensor_reduce(
                        out=t_sb[:, gs * J:ge * J], in_=pv,
                        axis=mybir.AxisListType.X, op=mybir.AluOpType.add)
                    a_ps = apsum.tile([128, 3 * J], f32, tag="a_ps")
                    nc.tensor.matmul(a_ps[:], lhsT=bpat_sb[:],
                                     rhs=t_sb[:, gs * J:ge * J],
                                     start=True, stop=True)
                    bsl = b_sb[:, gs * J:ge * J]
                    if first:
                        nc.vector.tensor_copy(out=bsl, in_=a_ps[:])
                    else:
                        nc.vector.tensor_tensor(out=bsl, in0=bsl, in1=a_ps[:],
                                                op=mybir.AluOpType.add)
                    nc.scalar.activation(
                        out=e_sb[:, gs * J:ge * J], in_=b_sb[:, gs * J:ge * J],
                        func=mybir.ActivationFunctionType.Exp)
                    if g3 == 0:
                        # hoist the next squash's Sqrt table load off-chain
                        nc.scalar.activation(
                            out=junk_sb[:], in_=e_sb[0:1, 0:1],
                            func=mybir.ActivationFunctionType.Sqrt)
                    nc.vector.tensor_reduce(
                        out=m_sb[:, gs:ge], in_=ev3[:, gs:ge, :],
                        axis=mybir.AxisListType.X, op=mybir.AluOpType.add)
                    nc.vector.reciprocal(out=m_sb[:, gs:ge],
                                         in_=m_sb[:, gs:ge])
                    rb = (m_sb[:, gs:ge].unsqueeze(-1)
                          .broadcast_to([128, 3, J]))
                    nc.vector.tensor_tensor(out=cv3[:, gs:ge, :],
                                            in0=ev3[:, gs:ge, :], in1=rb,
                                            op=mybir.AluOpType.mult)
                    if F16:
                        cwh4 = cwh_sb[:].rearrange("p (g j d) -> p g j d",
                                                   g=RG, j=J, d=D)
                        nc.vector.tensor_tensor(
                            out=cwh4[:, gs:ge], in0=wv4[:, gs:ge],
                            in1=cb4[:, gs:ge], op=mybir.AluOpType.mult)
                    else:
                        nc.vector.tensor_tensor(
                            out=cwv4[:, gs:ge], in0=wv4[:, gs:ge],
                            in1=cb4[:, gs:ge], op=mybir.AluOpType.mult)
                        cwc = cw_sb[:].rearrange("p (g f) -> p g f",
                                                 g=RG, f=JD)
                        nc.scalar.copy(out=cwh3[:, gs:ge, :],
                                       in_=cwc[:, gs:ge, :])
                        nc.vector.tensor_tensor(
                            out=cwl3[:, gs:ge, :], in0=cwc[:, gs:ge, :],
                            in1=cwh3[:, gs:ge, :],
                            op=mybir.AluOpType.subtract)
                    rh3 = cwh3
                    for g in range(gs, ge):
                        if F16:
                            sterms = ((xh[:, g, :], rh3[:, g, :]),)
                        else:
                            sterms = (
                                (xh[:, g, :], rh3[:, g, :]),
                                (xh[:, g, :], cwl3[:, g, :]),
                                (xl[:, g, :], rh3[:, g, :]),
                            )
                        for lhs, rhs in sterms:
                            nc.tensor.matmul(
                                s_ps[:], lhsT=lhs, rhs=rhs,
                                start=(i == 0), stop=(i == n_s - 1),
                                skip_group_check=True)
                            i += 1
                return s_ps

            for it in range(NUM_IT):
                if it == 0:
                    s_ps = s_matmuls(wh_sb, wl_sb)  # c uniform: 1/J in squash
                    scale = 1.0 / J
                else:
                    s_ps = fused_step(first=(it == 1))
                    scale = 1.0
                last = it == NUM_IT - 1
                nc.vector.tensor_copy(out=s_sb[:], in_=s_ps[:])
                nc.sync.dma_start(out=s_in[:], in_=s_sb[:])
                if it == 0 and not F16:
                    # f32 W (for the agreement/cw elementwise stages) is
                    # reconstructed from the shipped bf16 halves during AR1
                    nc.vector.tensor_tensor(out=w_sb[:], in0=wh_sb[:],
                                            in1=wl_sb[:],
                                            op=mybir.AluOpType.add)
                if last:
                    # final iteration: each core only needs its batch shard
                    # of s (the output is assembled on the host), so a
                    # ReduceScatter (half an AllReduce) suffices.
                    nc.gpsimd.collective_compute(
                        "ReduceScatter", mybir.AluOpType.add,
                        replica_groups=groups,
                        ins=[s_in[:]], outs=[s3_out[:]],
                    )
                    nc.sync.dma_start(out=s_sb[0:BS, :], in_=s3_out[:])
                    squash(scale, P=BS, split_v=False)
                    nc.sync.dma_start(out=out[:], in_=v_sb[0:BS, :])
                else:
                    nc.gpsimd.collective_compute(
                        "AllReduce", mybir.AluOpType.add,
                        replica_groups=groups,
                        ins=[s_in[:]], outs=[s_out[:]],
                    )
                    nc.sync.dma_start(out=s_sb[:], in_=s_out[:])
                    if it == 0:
                        # agreement-phase inputs; not needed until after the
                        # first AllReduce, so loaded in its shadow
                        nc.sync.dma_start(out=xth_sb[:], in_=xt_hi[:])
                        if not F16:
                            nc.sync.dma_start(out=xtl_sb[:], in_=xt_lo[:])
                        nc.sync.dma_start(out=bpat_sb[:], in_=bpat[:])
                    squash(scale)

    nc.finalize()
    return nc


def _split_hi_lo(a):
    hi = a.astype(ml_dtypes.bfloat16)
    lo = (a - hi.astype(np.float32)).astype(ml_dtypes.bfloat16)
    return hi, lo


def _prep_inputs(x, W):
    """Build per-core contiguous SBUF images."""
    x = np.ascontiguousarray(x, dtype=np.float32)
    W0 = np.ascontiguousarray(W.reshape(R, J, D, C), dtype=np.float32)
    # W0t[r, c, j, d]
    W0t = W0.transpose(0, 3, 1, 2)
    # (k, g, r16, c, j, d) -> (k, (r16, c), (g, j, d))
    w_img = np.ascontiguousarray(
        W0t.reshape(N_CORES, RG, 16, C, J, D)
        .transpose(0, 2, 3, 1, 4, 5)
        .reshape(N_CORES, 128, RG * JD)
    )
    xr = x.reshape(B, N_CORES, RG, 16, C)
    # (k, r16, c, g, b)
    xrc_img = np.ascontiguousarray(
        xr.transpose(1, 3, 4, 2, 0).reshape(N_CORES, 128, RG * B)
    )
    # (k, b, g, r16, c)
    xt_img = np.ascontiguousarray(
        xr.transpose(1, 0, 2, 3, 4).reshape(N_CORES, B, RG * 128)
    )
    p = np.arange(128)
    bpat = np.where((p[:, None] // C) == (p[None, :] // C), 1.0 / B, 0.0).astype(
        np.float32
    )
    return w_img, xrc_img, xt_img, bpat


def last_exec_time_ns():
    return _CACHE.get("exec_time_ns")


def kernel(input, W):
    from concourse.bass_utils import run_bass_kernel_spmd

    key = "nc_" + PRECISION
    if key not in _CACHE:
        _CACHE[key] = _build_bass(PRECISION)
    nc = _CACHE[key]

    w_img, xrc_img, xt_img, bpat = _prep_inputs(
        np.asarray(input), np.asarray(W)
    )
    if PRECISION == "f16":
        in_maps = [
            {
                "xrc16": xrc_img[k].astype(np.float16),
                "xt16": xt_img[k].astype(np.float16),
                "w16": w_img[k].astype(np.float16),
                "bpat": bpat,
            }
            for k in range(N_CORES)
        ]
    else:
        xrc_hi, xrc_lo = _split_hi_lo(xrc_img)
        xt_hi, xt_lo = _split_hi_lo(xt_img)
        w_hi, w_lo = _split_hi_lo(w_img)
        in_maps = [
            {
                "xrc_hi": xrc_hi[k],
                "xrc_lo": xrc_lo[k],
                "xt_hi": xt_hi[k],
                "xt_lo": xt_lo[k],
                "wh_d": w_hi[k],
                "wl_d": w_lo[k],
                "bpat": bpat,
            }
            for k in range(N_CORES)
        ]
    tdir = None
    if TRACE and TRACE_DIR:
        import tempfile

        tdir = tempfile.mkdtemp(prefix="run_", dir=TRACE_DIR)
    res = run_bass_kernel_spmd(
        nc, in_maps, list(range(N_CORES)), trace=TRACE, tmpdir=tdir
    )
    _CACHE["trace_dir"] = tdir
    _CACHE["exec_time_ns"] = res.exec_time_ns
    _CACHE["profile_json"] = res.profile_json
    # each core holds batch rows [16k, 16k+16) of the final v
    v = np.concatenate([res.results[k]["out"] for k in range(N_CORES)], axis=0)
    return np.ascontiguousarray(v.reshape(B, J, D, 1).astype(np.float32))



# revision 2
# speedup vs baseline: 1.0458x; 1.0458x over previous
"""DigitCaps dynamic-routing kernel for 8 TRN2 NeuronCores.

Reference computation (f32):
    u_hat[b,r,j,d] = sum_c W[r,j,d,c] * x[b,r,c]
    b_ij = 0
    for it in 1..3:
        c = softmax_j(b_ij)                       # [R, J]
        s[b,j,d] = sum_r c[r,j] u_hat[b,r,j,d]
        v = squash(s)                             # [B, J, D]
        b_ij += mean_b sum_d u_hat[b,r,j,d] v[b,j,d]
    return v[..., None]

Strategy: shard the R=1152 routes across 8 cores (144 each), full batch on
every core.  u_hat is never materialized; both big contractions go through
the rank-8 factorization:
    s[b,jd]   = sum_{rc} x[b,(rc)] * (c[r,j] W[(rc),(jd)])     (PE matmuls)
    a[r,j]    = sum_{cd} W[(rc),(jd)] * Q[(rc),(jd)]           (DVE + PE)
      where Q[(rc),(jd)] = (1/B) sum_b x[b,(rc)] v[b,(jd)]     (PE matmuls)
Per-iteration collective: one 80KB AllReduce of the partial s.
Host pre-arranges all device layouts so every DMA is contiguous.

All big matmuls run as compensated bf16 3-pass (hi*hi + lo*hi + hi*lo):
fp32 matmul on TRN2 is 4 cycles/row (LOW_HIGH dual pass) with slow
non-FWL weight loads; the bf16 triple runs >3x faster at ~1e-5 rel error.
"""

import sys

sys.path.insert(0, "/opt/trn_rl_repo")

import ml_dtypes
import numpy as np

B = 128          # batch
R = 1152         # num_routes
J = 10           # num_caps
D = 16           # caps_dim
C = 8            # caps_in
NUM_IT = 3
N_CORES = 8
RG = 9           # route groups per core (16 routes x 8 c = 128 partitions)
JD = J * D       # 160

_CACHE = {}

TRACE = False  # set True (e.g. from test.py) to capture HW profile/exec time
TRACE_DIR = None
PRECISION = "f16"  # "bf16x3" (safe, ~1e-5) or "f16" (fast, ~2e-4)


def _build_bass(precision="bf16x3"):
    import concourse.bacc as bacc
    import concourse.mybir as mybir
    from concourse.tile import TileContext

    f32 = mybir.dt.float32
    bf16 = mybir.dt.bfloat16
    f16 = mybir.dt.float16
    F16 = precision == "f16"
    mdt = f16 if F16 else bf16
    nc = bacc.Bacc(None, target_bir_lowering=False, num_devices=N_CORES)

    if F16:
        xrc_hi = nc.dram_tensor("xrc16", [128, RG * B], f16,
                                kind="ExternalInput")
        xt_hi = nc.dram_tensor("xt16", [B, RG * 128], f16,
                               kind="ExternalInput")
        w = nc.dram_tensor("w16", [128, RG * JD], f16, kind="ExternalInput")
        xrc_lo = xt_lo = None
    else:
        xrc_hi = nc.dram_tensor("xrc_hi", [128, RG * B], bf16,
                                kind="ExternalInput")
        xrc_lo = nc.dram_tensor("xrc_lo", [128, RG * B], bf16,
                                kind="ExternalInput")
        xt_hi = nc.dram_tensor("xt_hi", [B, RG * 128], bf16,
                               kind="ExternalInput")
        xt_lo = nc.dram_tensor("xt_lo", [B, RG * 128], bf16,
                               kind="ExternalInput")
        w = nc.dram_tensor("wh_d", [128, RG * JD], bf16, kind="ExternalInput")
        wl_d = nc.dram_tensor("wl_d", [128, RG * JD], bf16,
                              kind="ExternalInput")
    bpat = nc.dram_tensor("bpat", [128, 128], f32, kind="ExternalInput")
    BS = B // N_CORES  # final-iteration batch shard per core
    out = nc.dram_tensor("out", [BS, JD], f32, kind="ExternalOutput")

    # collective bounce buffers (DRAM)
    s_in = nc.dram_tensor("s_in", [B, JD], f32)
    s_out = nc.dram_tensor("s_out", [B, JD], f32, addr_space="Shared")
    s3_out = nc.dram_tensor("s3_out", [BS, JD], f32)

    groups = [list(range(N_CORES))]

    with TileContext(nc) as tc:
        with (
            tc.tile_pool(name="inp", bufs=1) as inp,
            tc.tile_pool(name="work", bufs=2) as work,
            tc.tile_pool(name="small", bufs=1) as small,
            tc.tile_pool(name="psum", bufs=2, space="PSUM") as psum,
            tc.tile_pool(name="apsum", bufs=3, space="PSUM") as apsum,
            tc.tile_pool(name="qpsum", bufs=3, space="PSUM") as qpsum,
        ):
            xrch_sb = inp.tile([128, RG * B], mdt, tag="xrch")
            xth_sb = inp.tile([B, RG * 128], mdt, tag="xth")
            bpat_sb = inp.tile([128, 128], f32, tag="bpat")
            if F16:
                xrcl_sb = xtl_sb = None
                w_sb = inp.tile([128, RG * JD], f16, tag="w")
                wh_sb = w_sb
                wl_sb = None
            else:
                xrcl_sb = inp.tile([128, RG * B], bf16, tag="xrcl")
                xtl_sb = inp.tile([B, RG * 128], bf16, tag="xtl")
                w_sb = small.tile([128, RG * JD], f32, tag="w")
                wh_sb = inp.tile([128, RG * JD], bf16, tag="wh")
                wl_sb = inp.tile([128, RG * JD], bf16, tag="wl")

            # per-group-chunk loads of xrc/w so iteration-1 matmuls start
            # while the rest of the inputs stream in; xt/bpat are not needed
            # until after the first AllReduce and get emitted later.
            for g3 in range(3):  # 3-group chunks: larger DMA runs, still piped
                gs, ge = g3 * 3, (g3 + 1) * 3
                nc.sync.dma_start(
                    out=xrch_sb[:, gs * B:ge * B],
                    in_=xrc_hi[:, gs * B:ge * B])
                if not F16:
                    nc.sync.dma_start(
                        out=xrcl_sb[:, gs * B:ge * B],
                        in_=xrc_lo[:, gs * B:ge * B])
                nc.sync.dma_start(
                    out=wh_sb[:, gs * JD:ge * JD],
                    in_=w[:, gs * JD:ge * JD])
                if not F16:
                    nc.sync.dma_start(
                        out=wl_sb[:, gs * JD:ge * JD],
                        in_=wl_d[:, gs * JD:ge * JD])

            junk_sb = small.tile([1, 1], f32, tag="junk")
            nc.vector.memset(junk_sb[:], 1.0)
            # pre-load the ACT Sqrt table while the input DMA streams, so the
            # first squash's Sqrt doesn't pay the 1.3us table load on-chain
            nc.scalar.activation(out=junk_sb[:], in_=junk_sb[:],
                                 func=mybir.ActivationFunctionType.Sqrt)

            cw_sb = small.tile([128, RG * JD], f32, tag="cw")
            cwh_sb = small.tile([128, RG * JD], mdt, tag="cwh")
            cwl_sb = (None if F16
                      else small.tile([128, RG * JD], bf16, tag="cwl"))
            b_sb = small.tile([128, RG * J], f32, tag="bij")
            c_sb = small.tile([128, RG * J], f32, tag="cij")
            t_sb = small.tile([128, RG * J], f32, tag="t")
            v_sb = small.tile([B, JD], f32, tag="v")
            vh_sb = small.tile([B, JD], mdt, tag="vh")
            vl_sb = None if F16 else small.tile([B, JD], bf16, tag="vl")
            s_sb = small.tile([B, JD], f32, tag="s")
            # softmax/squash temporaries
            m_sb = small.tile([128, RG], f32, tag="m")
            e_sb = small.tile([128, RG * J], f32, tag="e")
            sqn_sb = small.tile([B, J], f32, tag="sqn")
            rt_sb = small.tile([B, J], f32, tag="rt")
            den_sb = small.tile([B, J], f32, tag="den")
            fac_sb = small.tile([B, J], f32, tag="fac")
            s2_sb = small.tile([B, JD], f32, tag="s2")

            def softmax():
                """c_sb = softmax_j(b_sb), per (partition, g)."""
                cv = c_sb[:].rearrange("p (g j) -> p g j", g=RG, j=J)
                ev = e_sb[:].rearrange("p (g j) -> p g j", g=RG, j=J)
                # logits are bounded (|b| < ~10): exp without max-shift
                nc.scalar.activation(
                    out=e_sb[:], in_=b_sb[:], func=mybir.ActivationFunctionType.Exp
                )
                # hoist the next squash's Sqrt table load off the chain
                nc.scalar.activation(out=junk_sb[:], in_=e_sb[0:1, 0:1],
                                     func=mybir.ActivationFunctionType.Sqrt)
                nc.vector.tensor_reduce(
                    out=m_sb[:], in_=ev, axis=mybir.AxisListType.X,
                    op=mybir.AluOpType.add,
                )
                nc.vector.reciprocal(out=m_sb[:], in_=m_sb[:])
                rb = m_sb[:].unsqueeze(-1).broadcast_to([128, RG, J])
                nc.vector.tensor_tensor(
                    out=cv, in0=ev, in1=rb, op=mybir.AluOpType.mult
                )

            def compute_cw():
                """cw = w * c (broadcast over d), split hi/lo bf16."""
                wv = w_sb[:].rearrange("p (g j d) -> p g j d", g=RG, j=J, d=D)
                cwv = cw_sb[:].rearrange("p (g j d) -> p g j d", g=RG, j=J, d=D)
                cb = (
                    c_sb[:]
                    .rearrange("p (g j) -> p g j", g=RG, j=J)
                    .unsqueeze(-1)
                    .broadcast_to([128, RG, J, D])
                )
                if F16:
                    cwhv = cwh_sb[:].rearrange("p (g j d) -> p g j d",
                                               g=RG, j=J, d=D)
                    nc.vector.tensor_tensor(out=cwhv, in0=wv, in1=cb,
                                            op=mybir.AluOpType.mult)
                else:
                    # chunked by 3-group slices: the s-matmuls on early
                    # groups start while later chunks are still built
                    cwhc = cwh_sb[:].rearrange("p (g f) -> p g f", g=RG, f=JD)
                    cwlc = cwl_sb[:].rearrange("p (g f) -> p g f", g=RG, f=JD)
                    cwc = cw_sb[:].rearrange("p (g f) -> p g f", g=RG, f=JD)
                    for g3 in range(3):
                        gs, ge = g3 * 3, (g3 + 1) * 3
                        nc.vector.tensor_tensor(
                            out=cwv[:, gs:ge], in0=wv[:, gs:ge],
                            in1=cb[:, gs:ge], op=mybir.AluOpType.mult)
                        nc.scalar.copy(out=cwhc[:, gs:ge, :],
                                       in_=cwc[:, gs:ge, :])
                        nc.vector.tensor_tensor(
                            out=cwlc[:, gs:ge, :], in0=cwc[:, gs:ge, :],
                            in1=cwhc[:, gs:ge, :],
                            op=mybir.AluOpType.subtract)

            def s_matmuls(rhsh_sb, rhsl_sb):
                """PSUM-accumulated compensated product sum over all groups."""
                s_ps = psum.tile([128, JD], f32, tag="s_ps")
                xh = xrch_sb[:].rearrange("p (g b) -> p g b", g=RG, b=B)
                rh = rhsh_sb[:].rearrange("p (g f) -> p g f", g=RG, f=JD)
                if F16:
                    terms = lambda g: ((xh[:, g, :], rh[:, g, :]),)
                    n = RG
                else:
                    xl = xrcl_sb[:].rearrange("p (g b) -> p g b", g=RG, b=B)
                    rl = rhsl_sb[:].rearrange("p (g f) -> p g f", g=RG, f=JD)
                    terms = lambda g: (
                        (xh[:, g, :], rh[:, g, :]),
                        (xh[:, g, :], rl[:, g, :]),
                        (xl[:, g, :], rh[:, g, :]),
                    )
                    n = 3 * RG
                i = 0
                for g in range(RG):
                    for lhs, rhs in terms(g):
                        nc.tensor.matmul(
                            s_ps[:], lhsT=lhs, rhs=rhs,
                            start=(i == 0), stop=(i == n - 1),
                        )
                        i += 1
                return s_ps

            def squash(scale, P=B, split_v=True):
                """v_sb[:P] = squash(scale * s_sb[:P]); optional hi/lo split."""
                s_ap = s_sb[0:P, :]
                v_ap = v_sb[0:P, :]
                vv = v_ap.rearrange("b (j d) -> b j d", j=J, d=D)
                s2v = s2_sb[0:P, :].rearrange("b (j d) -> b j d", j=J, d=D)
                # s <- scale * s  (ACT Copy, no table)
                if scale != 1.0:
                    nc.scalar.mul(out=s_ap, in_=s_ap, mul=float(scale))
                nc.vector.tensor_tensor(out=s2_sb[0:P, :], in0=s_ap, in1=s_ap,
                                        op=mybir.AluOpType.mult)
                nc.vector.tensor_reduce(
                    out=sqn_sb[0:P, :], in_=s2v, axis=mybir.AxisListType.X,
                    op=mybir.AluOpType.add,
                )
                # factor = sqrt(sqn) / (1 + sqn)
                nc.scalar.activation(
                    out=rt_sb[0:P, :], in_=sqn_sb[0:P, :],
                    func=mybir.ActivationFunctionType.Sqrt,
                )
                if split_v:
                    # hoist the next softmax's Exp table load off the chain
                    nc.scalar.activation(out=junk_sb[:], in_=rt_sb[0:1, 0:1],
                                         func=mybir.ActivationFunctionType.Exp)
                nc.vector.tensor_scalar_add(out=den_sb[0:P, :],
                                            in0=sqn_sb[0:P, :], scalar1=1.0)
                nc.vector.reciprocal(out=den_sb[0:P, :], in_=den_sb[0:P, :])
                nc.vector.tensor_tensor(out=fac_sb[0:P, :], in0=rt_sb[0:P, :],
                                        in1=den_sb[0:P, :],
                                        op=mybir.AluOpType.mult)
                fb = fac_sb[0:P, :].unsqueeze(-1).broadcast_to([P, J, D])
                sv = s_ap.rearrange("b (j d) -> b j d", j=J, d=D)
                nc.vector.tensor_tensor(out=vv, in0=sv, in1=fb,
                                        op=mybir.AluOpType.mult)
                if split_v:
                    vhv = vh_sb[:].rearrange("b (j d) -> b j d", j=J, d=D)
                    nc.vector.tensor_tensor(out=vhv, in0=sv, in1=fb,
                                            op=mybir.AluOpType.mult)
                    if not F16:
                        nc.vector.tensor_tensor(out=vl_sb[:], in0=v_sb[:],
                                                in1=vh_sb[:],
                                                op=mybir.AluOpType.subtract)

            def agreement(first):
                """t_sb[p,(g,j)] = sum_d W*Q;  b_sb += bpat^T @ t (c-sum,
                broadcast over c partitions, * 1/B folded into bpat)."""
                wv = w_sb[:].rearrange("p (g f) -> p g f", g=RG, f=JD)
                tv = t_sb[:].rearrange("p (g j) -> p g j", g=RG, j=J)
                for g in range(RG):
                    q_ps = qpsum.tile([128, JD], f32, tag="q_ps")
                    if F16:
                        qterms = ((xth_sb[:, g * 128:(g + 1) * 128],
                                   vh_sb[:]),)
                    else:
                        qterms = (
                            (xth_sb[:, g * 128:(g + 1) * 128], vh_sb[:]),
                            (xth_sb[:, g * 128:(g + 1) * 128], vl_sb[:]),
                            (xtl_sb[:, g * 128:(g + 1) * 128], vh_sb[:]),
                        )
                    for i, (lhs, rhs) in enumerate(qterms):
                        nc.tensor.matmul(q_ps[:], lhsT=lhs, rhs=rhs,
                                         start=(i == 0),
                                         stop=(i == len(qterms) - 1))
                    prod = work.tile([128, JD], f32, tag="prod")
                    pv = prod[:].rearrange("p (j d) -> p j d", j=J, d=D)
                    nc.vector.tensor_tensor(
                        out=prod[:], in0=wv[:, g, :],
                        in1=q_ps[:], op=mybir.AluOpType.mult,
                    )
                    nc.vector.tensor_reduce(
                        out=tv[:, g, :], in_=pv, axis=mybir.AxisListType.X,
                        op=mybir.AluOpType.add,
                    )
                a_ps = apsum.tile([128, RG * J], f32, tag="a_ps")
                nc.tensor.matmul(a_ps[:], lhsT=bpat_sb[:], rhs=t_sb[:],
                                 start=True, stop=True)
                if first:
                    nc.vector.tensor_copy(out=b_sb[:], in_=a_ps[:])
                else:
                    nc.vector.tensor_tensor(out=b_sb[:], in0=b_sb[:],
                                            in1=a_ps[:],
                                            op=mybir.AluOpType.add)

            def fused_step(first):
                """Agreement + b-update + softmax + cw + s-matmuls, pipelined
                in 3-group chunks: the a-matmul/softmax/cw/s-MMs of early
                chunks run while later chunks' Q-matmuls are still going."""
                s_ps = psum.tile([128, JD], f32, tag="s_ps")
                wv = w_sb[:].rearrange("p (g f) -> p g f", g=RG, f=JD)
                wv4 = w_sb[:].rearrange("p (g j d) -> p g j d", g=RG, j=J, d=D)
                tv = t_sb[:].rearrange("p (g j) -> p g j", g=RG, j=J)
                ev3 = e_sb[:].rearrange("p (g j) -> p g j", g=RG, j=J)
                cv3 = c_sb[:].rearrange("p (g j) -> p g j", g=RG, j=J)
                cwv4 = cw_sb[:].rearrange("p (g j d) -> p g j d",
                                          g=RG, j=J, d=D)
                cwh3 = cwh_sb[:].rearrange("p (g f) -> p g f", g=RG, f=JD)
                cb4 = (c_sb[:].rearrange("p (g j) -> p g j", g=RG, j=J)
                       .unsqueeze(-1).broadcast_to([128, RG, J, D]))
                xh = xrch_sb[:].rearrange("p (g b) -> p g b", g=RG, b=B)
                if not F16:
                    cwl3 = cwl_sb[:].rearrange("p (g f) -> p g f", g=RG, f=JD)
                    xl = xrcl_sb[:].rearrange("p (g b) -> p g b", g=RG, b=B)
                    n_s = 3 * RG
                else:
                    n_s = RG
                i = 0
                for g3 in range(3):
                    gs, ge = 3 * g3, 3 * (g3 + 1)
                    # one wide PSUM tile per chunk: the W*Q multiply and
                    # d-reduce then run as 2 DVE ops instead of 6
                    q_ps = qpsum.tile([128, 3 * JD], f32, tag="q_ps")
                    for g in range(gs, ge):
                        off = (g - gs) * JD
                        if F16:
                            qterms = ((xth_sb[:, g * 128:(g + 1) * 128],
                                       vh_sb[:]),)
                        else:
                            qterms = (
                                (xth_sb[:, g * 128:(g + 1) * 128], vh_sb[:]),
                                (xth_sb[:, g * 128:(g + 1) * 128], vl_sb[:]),
                                (xtl_sb[:, g * 128:(g + 1) * 128], vh_sb[:]),
                            )
                        for qi, (lhs, rhs) in enumerate(qterms):
                            nc.tensor.matmul(q_ps[:, off:off + JD],
                                             lhsT=lhs, rhs=rhs,
                                             start=(qi == 0),
                                             stop=(qi == len(qterms) - 1),
                                             skip_group_check=True)
                    prod = work.tile([128, 3 * JD], f32, tag="prod")
                    pv = prod[:].rearrange("p (gj d) -> p gj d",
                                           gj=3 * J, d=D)
                    nc.vector.tensor_tensor(
                        out=prod[:], in0=w_sb[:, gs * JD:ge * JD],
                        in1=q_ps[:], op=mybir.AluOpType.mult)
                    nc.vector.tensor_reduce(
                        out=t_sb[:, gs * J:ge * J], in_=pv,
                        axis=mybir.AxisListType.X, op=mybir.AluOpType.add)
                    a_ps = apsum.tile([128, 3 * J], f32, tag="a_ps")
                    nc.tensor.matmul(a_ps[:], lhsT=bpat_sb[:],
                                     rhs=t_sb[:, gs * J:ge * J],
                                     start=True, stop=True)
                    bsl = b_sb[:, gs * J:ge * J]
                    if first:
                        nc.vector.tensor_copy(out=bsl, in_=a_ps[:])
                    else:
                        nc.vector.tensor_tensor(out=bsl, in0=bsl, in1=a_ps[:],
                                                op=mybir.AluOpType.add)
                    nc.scalar.activation(
                        out=e_sb[:, gs * J:ge * J], in_=b_sb[:, gs * J:ge * J],
                        func=mybir.ActivationFunctionType.Exp)
                    if g3 == 0:
                        # hoist the next squash's Sqrt table load off-chain
                        nc.scalar.activation(
                            out=junk_sb[:], in_=e_sb[0:1, 0:1],
                            func=mybir.ActivationFunctionType.Sqrt)
                    nc.vector.tensor_reduce(
                        out=m_sb[:, gs:ge], in_=ev3[:, gs:ge, :],
                        axis=mybir.AxisListType.X, op=mybir.AluOpType.add)
                    nc.vector.reciprocal(out=m_sb[:, gs:ge],
                                         in_=m_sb[:, gs:ge])
                    rb = (m_sb[:, gs:ge].unsqueeze(-1)
                          .broadcast_to([128, 3, J]))
                    nc.vector.tensor_tensor(out=cv3[:, gs:ge, :],
                                            in0=ev3[:, gs:ge, :], in1=rb,
                                            op=mybir.AluOpType.mult)
                    if F16:
                        cwh4 = cwh_sb[:].rearrange("p (g j d) -> p g j d",
                                                   g=RG, j=J, d=D)
                        nc.vector.tensor_tensor(
                            out=cwh4[:, gs:ge], in0=wv4[:, gs:ge],
                            in1=cb4[:, gs:ge], op=mybir.AluOpType.mult)
                    else:
                        nc.vector.tensor_tensor(
                            out=cwv4[:, gs:ge], in0=wv4[:, gs:ge],
                            in1=cb4[:, gs:ge], op=mybir.AluOpType.mult)
                        cwc = cw_sb[:].rearrange("p (g f) -> p g f",
                                                 g=RG, f=JD)
                        nc.scalar.copy(out=cwh3[:, gs:ge, :],
                                       in_=cwc[:, gs:ge, :])
                        nc.vector.tensor_tensor(
                            out=cwl3[:, gs:ge, :], in0=cwc[:, gs:ge, :],
                            in1=cwh3[:, gs:ge, :],
                            op=mybir.AluOpType.subtract)
                    rh3 = cwh3
                    for g in range(gs, ge):
                        if F16:
                            sterms = ((xh[:, g, :], rh3[:, g, :]),)
                        else:
                            sterms = (
                                (xh[:, g, :], rh3[:, g, :]),
                                (xh[:, g, :], cwl3[:, g, :]),
                                (xl[:, g, :], rh3[:, g, :]),
                            )
                        for lhs, rhs in sterms:
                            nc.tensor.matmul(
                                s_ps[:], lhsT=lhs, rhs=rhs,
                                start=(i == 0), stop=(i == n_s - 1),
                                skip_group_check=True)
                            i += 1
                return s_ps

            for it in range(NUM_IT):
                if it == 0:
                    s_ps = s_matmuls(wh_sb, wl_sb)  # c uniform: 1/J in squash
                    scale = 1.0 / J
                else:
                    s_ps = fused_step(first=(it == 1))
                    scale = 1.0
                last = it == NUM_IT - 1
                nc.vector.tensor_copy(out=s_sb[:], in_=s_ps[:])
                nc.sync.dma_start(out=s_in[:], in_=s_sb[:])
                if it == 0 and not F16:
                    # f32 W (for the agreement/cw elementwise stages) is
                    # reconstructed from the shipped bf16 halves during AR1
                    nc.vector.tensor_tensor(out=w_sb[:], in0=wh_sb[:],
                                            in1=wl_sb[:],
                                            op=mybir.AluOpType.add)
                if last:
                    # final iteration: each core only needs its batch shard
                    # of s (the output is assembled on the host), so a
                    # ReduceScatter (half an AllReduce) suffices.
                    nc.gpsimd.collective_compute(
                        "ReduceScatter", mybir.AluOpType.add,
                        replica_groups=groups,
                        ins=[s_in[:]], outs=[s3_out[:]],
                    )
                    nc.sync.dma_start(out=s_sb[0:BS, :], in_=s3_out[:])
                    squash(scale, P=BS, split_v=False)
                    nc.sync.dma_start(out=out[:], in_=v_sb[0:BS, :])
                else:
                    nc.gpsimd.collective_compute(
                        "AllReduce", mybir.AluOpType.add,
                        replica_groups=groups,
                        ins=[s_in[:]], outs=[s_out[:]],
                    )
                    nc.sync.dma_start(out=s_sb[:], in_=s_out[:])
                    if it == 0:
                        # agreement-phase inputs; not needed until after the
                        # first AllReduce, so loaded in its shadow
                        nc.sync.dma_start(out=xth_sb[:], in_=xt_hi[:])
                        if not F16:
                            nc.sync.dma_start(out=xtl_sb[:], in_=xt_lo[:])
                        nc.sync.dma_start(out=bpat_sb[:], in_=bpat[:])
                    squash(scale)

    nc.finalize()
    return nc


def _split_hi_lo(a):
    hi = a.astype(ml_dtypes.bfloat16)
    lo = (a - hi.astype(np.float32)).astype(ml_dtypes.bfloat16)
    return hi, lo


def _prep_inputs(x, W):
    """Build per-core contiguous SBUF images."""
    x = np.ascontiguousarray(x, dtype=np.float32)
    W0 = np.ascontiguousarray(W.reshape(R, J, D, C), dtype=np.float32)
    # W0t[r, c, j, d]
    W0t = W0.transpose(0, 3, 1, 2)
    # (k, g, r16, c, j, d) -> (k, (r16, c), (g, j, d))
    w_img = np.ascontiguousarray(
        W0t.reshape(N_CORES, RG, 16, C, J, D)
        .transpose(0, 2, 3, 1, 4, 5)
        .reshape(N_CORES, 128, RG * JD)
    )
    xr = x.reshape(B, N_CORES, RG, 16, C)
    # (k, r16, c, g, b)
    xrc_img = np.ascontiguousarray(
        xr.transpose(1, 3, 4, 2, 0).reshape(N_CORES, 128, RG * B)
    )
    # (k, b, g, r16, c)
    xt_img = np.ascontiguousarray(
        xr.transpose(1, 0, 2, 3, 4).reshape(N_CORES, B, RG * 128)
    )
    p = np.arange(128)
    bpat = np.where((p[:, None] // C) == (p[None, :] // C), 1.0 / B, 0.0).astype(
        np.float32
    )
    return w_img, xrc_img, xt_img, bpat


def last_exec_time_ns():
    return _CACHE.get("exec_time_ns")


def kernel(input, W):
    from concourse.bass_utils import run_bass_kernel_spmd

    key = "nc_" + PRECISION
    if key not in _CACHE:
        _CACHE[key] = _build_bass(PRECISION)
    nc = _CACHE[key]

    w_img, xrc_img, xt_img, bpat = _prep_inputs(
        np.asarray(input), np.asarray(W)
    )
    if PRECISION == "f16":
        in_maps = [
            {
                "xrc16": xrc_img[k].astype(np.float16),
                "xt16": xt_img[k].astype(np.float16),
                "w16": w_img[k].astype(np.float16),
                "bpat": bpat,
            }
            for k in range(N_CORES)
        ]
    else:
        xrc_hi, xrc_lo = _split_hi_lo(xrc_img)
        xt_hi, xt_lo = _split_hi_lo(xt_img)
        w_hi, w_lo = _split_hi_lo(w_img)
        in_maps = [
            {
                "xrc_hi": xrc_hi[k],
                "xrc_lo": xrc_lo[k],
                "xt_hi": xt_hi[k],
                "xt_lo": xt_lo[k],
                "wh_d": w_hi[k],
                "wl_d": w_lo[k],
                "bpat": bpat,
            }
            for k in range(N_CORES)
        ]
    tdir = None
    if TRACE and TRACE_DIR:
        import tempfile

        tdir = tempfile.mkdtemp(prefix="run_", dir=TRACE_DIR)
    res = run_bass_kernel_spmd(
        nc, in_maps, list(range(N_CORES)), trace=TRACE, tmpdir=tdir
    )
    _CACHE["trace_dir"] = tdir
    _CACHE["exec_time_ns"] = res.exec_time_ns
    _CACHE["profile_json"] = res.profile_json
    # each core holds batch rows [16k, 16k+16) of the final v
    v = np.concatenate([res.results[k]["out"] for k in range(N_CORES)], axis=0)
    return np.ascontiguousarray(v.reshape(B, J, D, 1).astype(np.float32))

